# revision 10
# baseline (speedup 1.0000x reference)
"""Causal attention kernel for Trainium2 (8 NeuronCores, SPMD over heads).

Problem: B=4, H=16, S=2048, D=64, fp32.
  scores = Q @ K^T / sqrt(64); causal mask; softmax (the reference's global-max
  shift cancels exactly); out = attn @ V.

Distribution: B*H = 64 heads -> 8 heads per core, embarrassingly parallel.

Per-core algorithm (per head, one full q-pass):
  - Host pre-transposes Q,K to [D,S] per head; V gets a ones-column appended.
  - QK: scoresT[k,q] = sum_d K[k,d] Q[q,d] with K-tiles stationary and Q^T
    streaming in 512-col chunks. Contraction is D=64, so even k-tiles use PE
    rows 0-63 and odd k-tiles rows 64-127 (row packing -> pairs of matmuls
    run concurrently). Scores land in rotating 1-bank PSUM tiles.
  - exp: split across ScalarE and VectorE to double elementwise throughput.
    The first 512-chunk of each k-row (contains the diagonal block) uses
    ScalarE's exact LUT exp; remaining chunks go to whichever of ScalarE /
    VectorE has less queued work.  VectorE computes exp via a one-instruction
    Schraudolph bit-trick: int16(round(s*A + B)) reinterpreted as fp16 equals
    exp(s/8) to ~2% (the systematic part cancels in softmax; end-to-end
    contribution ~0.5% rel err).  Causal masking of the diagonal block is a
    gpsimd multiply by a triangular 0/1 matrix (post-exp).
  - PV: at-stationary matmuls: acc_q[q,0:65] += at_k[:,128q:128q+128]^T @
    [V|1]-tile. Column 64 is the softmax denominator for free. Accumulators
    for all 16 q-tiles pack into 3 PSUM banks (65 f32 each); with the
    bank-wide has_written clear of start=True, only the first write into each
    bank uses start=True.  Output is directly q-major: no PE transposes.
  - Normalize: per-bank gather of rowsums -> DVE reciprocal -> per-partition
    scalar multiply on ScalarE/VectorE (balanced), DMA out [128,64] f32 tiles.
"""

import math
import sys

import numpy as np

if "/opt/trn_rl_repo" not in sys.path:
    sys.path.insert(0, "/opt/trn_rl_repo")

B, H, S, D = 4, 16, 2048, 64
N_CORES = 8
HEADS_PER_CORE = (B * H) // N_CORES  # 8
CHUNK = 512  # QK moving-chunk / PSUM score-tile width (1 bank)

# Schraudolph exp-to-fp16 constants: int16(s*A16 + B16) bit-viewed as fp16
# approximates exp(s/8).  A16 = 1024 * 0.125 * log2(e); B16 = 15360 - C with
# C = 59 centering the sawtooth error (calibrated numerically; HW convert is
# round-half-even with saturation).
A16 = 1024.0 * 0.125 * math.log2(math.e)
B16 = 15360.0 - 59.0

import os
FORCE_ENG = os.environ.get("KM_FORCE_ENG", "")  # "s"/"d" to force exp engine

# q-tile -> (acc tile index, region index): accs 0/1 hold 6 q-tiles, acc 2
# holds 4; 390/390/260 f32 per partition, one PSUM bank each.
ACC_SPLIT = (6, 6, 4)


def _acc_loc(q):
    if q < 6:
        return 0, q
    if q < 12:
        return 1, q - 6
    return 2, q - 12


def _chunks(lo, hi):
    """Split [lo, hi) at absolute multiples of CHUNK."""
    out = []
    c = lo
    while c < hi:
        w = min(hi, (c // CHUNK + 1) * CHUNK) - c
        out.append((c, w))
        c += w
    return out


def build_attention(tc, outs, ins, n_heads=HEADS_PER_CORE, s=S):
    import concourse.bass as bass
    import concourse.mybir as mybir

    nc = tc.nc
    f32 = mybir.dt.float32
    f16 = mybir.dt.float16
    i16 = mybir.dt.int16
    Exp = mybir.ActivationFunctionType.Exp
    Mult = mybir.AluOpType.mult
    Add = mybir.AluOpType.add

    qt_d, kt_d, v_d = ins["qt"], ins["kt"], ins["v"]
    tri_d = ins["ctri"]
    ot_d = outs["ot"]

    n_kt = s // 128  # 16 k-tiles
    n_pairs = n_kt // 2

    # rough per-engine queued-ns estimators for load balancing
    est = {"s": 0.0, "d": 0.0}

    def s_cost(w):
        return (172.0 + w) / 1.2

    def d_cost(w):
        return (120.0 + w) / 0.96

    with (
        tc.tile_pool(name="consts", bufs=1) as cpool,
        tc.tile_pool(name="qpool", bufs=2) as qpool,
        tc.tile_pool(name="kpool", bufs=2) as kpool,
        tc.tile_pool(name="vpool", bufs=2) as vpool,
        tc.tile_pool(name="atpool", bufs=2) as atpool,
        tc.tile_pool(name="ofpool", bufs=8) as ofpool,
        tc.tile_pool(name="rpool", bufs=2) as rpool,
        tc.tile_pool(name="scpool", bufs=5, space="PSUM") as scpool,
        tc.tile_pool(name="accpool", bufs=1, space="PSUM") as accpool,
    ):
        c_tri = cpool.tile([128, 128], f16, tag="ctri")

        for h in range(n_heads):
            # ---- input loads (host pre-arranged to contiguous per-partition
            # lines; QK needs kt/qt first, PV needs vx only later) ----
            kt2 = kpool.tile([128, s // 2], f16, tag="kt2", name=f"kt2_{h}")
            kt2_v = kt2.rearrange("p (t c) -> p t c", c=128)
            nc.sync.dma_start(kt2[:], kt_d[h])
            qt2 = qpool.tile([128, s], f16, tag="qt2", name=f"qt2_{h}")
            nc.sync.dma_start(qt2[0:64, :], qt_d[h])
            nc.sync.dma_start(qt2[64:128, :], qt_d[h])
            vx = vpool.tile([128, n_kt * 65], f16, tag="vx", name=f"vx_{h}")
            vx_v = vx.rearrange("p (t c) -> p t c", c=65)
            nc.sync.dma_start(vx[:], v_d[h])
            if h == 0:
                nc.sync.dma_start(c_tri[:], tri_d[:])

            at_tiles = {}
            for k in range(n_kt):
                at_tiles[k] = atpool.tile(
                    [128, s - 128 * k], f16, tag=f"at{k}", name=f"at_{h}_{k}"
                )

            accs = [
                accpool.tile(
                    [128, 65 * n], f32, tag=f"acc{i}", name=f"acc{i}_{h}"
                )
                for i, n in enumerate(ACC_SPLIT)
            ]

            def qk_op(k, c0, w, p, h=h, qt2=qt2, kt2_v=kt2_v, at_tiles=at_tiles):
                """One QK chunk: matmul into a score slot + exp into at."""
                half = k % 2
                sc = scpool.tile([128, CHUNK], f32, tag="sc", name=f"sc_{h}_{k}_{c0}")
                nc.tensor.matmul(
                    sc[:, 0:w],
                    kt2_v[64 * half : 64 * half + 64, p],
                    qt2[64 * half : 64 * half + 64, c0 : c0 + w],
                    start=True,
                    stop=True,
                    skip_group_check=True,
                )
                rel = c0 - 128 * k
                first = rel == 0
                if first or FORCE_ENG == "s":
                    eng = "s"
                elif FORCE_ENG == "d":
                    eng = "d"
                else:
                    eng = "s" if est["s"] <= est["d"] else "d"
                if eng == "s":
                    nc.scalar.activation(
                        at_tiles[k][:, rel : rel + w], sc[:, 0:w], Exp, scale=0.125
                    )
                    est["s"] += s_cost(w)
                else:
                    nc.vector.tensor_scalar(
                        at_tiles[k].bitcast(i16)[:, rel : rel + w],
                        sc[:, 0:w],
                        A16,
                        B16,
                        Mult,
                        Add,
                    )
                    est["d"] += d_cost(w)
                if first:
                    # causal mask of the diagonal block (post-exp)
                    nc.gpsimd.tensor_tensor(
                        at_tiles[k][:, 0:128], at_tiles[k][:, 0:128], c_tri[:], Mult
                    )

            def qk_ops(p):
                ke, ko = 2 * p, 2 * p + 1
                ch = {ke: _chunks(128 * ke, s), ko: _chunks(128 * ko, s)}
                ops = []
                for ci in range(max(len(ch[ke]), len(ch[ko]))):
                    for k in (ke, ko):
                        if ci < len(ch[k]):
                            c0, w = ch[k][ci]
                            ops.append(lambda k=k, c0=c0, w=w: qk_op(k, c0, w, p))
                return ops

            def pv_op(k, q, at_tiles=at_tiles, accs=accs, vx_v=vx_v):
                ai, ri = _acc_loc(q)
                off = 128 * (q - k)
                nc.tensor.matmul(
                    accs[ai][:, 65 * ri : 65 * ri + 65],
                    at_tiles[k][:, off : off + 128],
                    vx_v[:, k, :],
                    start=(k == 0 and ri == 0),
                    stop=(k == q),
                    skip_group_check=True,
                )

            def pv_ops(p):
                ops = []
                for k in (2 * p, 2 * p + 1):
                    for q in range(k, n_kt):
                        ops.append(lambda k=k, q=q: pv_op(k, q))
                return ops

            def emit_norm(ai, js, h=h, accs=accs):
                """Normalize q-tiles (sum(ACC_SPLIT[:ai]) + j for j in js)."""
                q0 = sum(ACC_SPLIT[:ai])
                n = len(js)
                j0 = js[0]
                acc_v = accs[ai].rearrange("p (j c) -> p j c", c=65)
                rsum = rpool.tile(
                    [128, n], f32, tag=f"rsum{ai}_{j0}", name=f"rsum{ai}_{j0}_{h}"
                )
                nc.vector.tensor_copy(rsum[:], acc_v[:, j0 : j0 + n, 64:65])
                rcp = rpool.tile(
                    [128, n], f32, tag=f"rcp{ai}_{j0}", name=f"rcp{ai}_{j0}_{h}"
                )
                nc.vector.reciprocal(rcp[:], rsum[:])
                est["d"] += d_cost(n) + d_cost(8 * n)
                for i, j in enumerate(js):
                    q = q0 + j
                    of = ofpool.tile([128, 64], f32, tag="of", name=f"of_{h}_{q}")
                    if est["s"] <= est["d"]:
                        nc.scalar.mul(of[:], acc_v[:, j, 0:64], rcp[:, i : i + 1])
                        est["s"] += s_cost(64)
                    else:
                        nc.vector.tensor_scalar(
                            of[:], acc_v[:, j, 0:64], rcp[:, i : i + 1], None, Mult
                        )
                        est["d"] += d_cost(64)
                    nc.sync.dma_start(ot_d[h, 128 * q : 128 * q + 128, :], of[:])

            def interleave(qk, pv):
                """Emit QK chunk ops with PV ops spread between them."""
                if not qk:
                    for op in pv:
                        op()
                    return
                done = 0
                for i, op in enumerate(qk):
                    op()
                    want = ((i + 1) * len(pv)) // len(qk)
                    while done < want:
                        pv[done]()
                        done += 1

            for p in range(n_pairs):
                interleave(qk_ops(p), pv_ops(p - 1) if p >= 1 else [])
                if p == 3:
                    emit_norm(0, [0, 1, 2, 3, 4, 5])  # final after PV(pair 2)
                if p == 6:
                    emit_norm(1, [0, 1, 2, 3, 4, 5])  # final after PV(pair 5)
            interleave([], pv_ops(n_pairs - 1))
            emit_norm(2, [0, 1])
            emit_norm(2, [2, 3])


def _make_consts():
    kk, qq = np.meshgrid(np.arange(128), np.arange(128), indexing="ij")
    tri = (kk <= qq).astype(np.float16)  # keep-mask for the diagonal block
    return tri


def _pack_kt(K):
    """[nh, S, D] -> [nh, 128, S//2]: even k-tiles in partitions 0-63 (d=p),
    odd k-tiles in partitions 64-127 (d=p-64); 128-col tiles concatenated."""
    nh = K.shape[0]
    kt = K.astype(np.float16).transpose(0, 2, 1)  # [nh, D, S]
    kt = kt.reshape(nh, D, S // 256, 2, 128)
    return np.ascontiguousarray(
        np.concatenate([kt[:, :, :, 0, :], kt[:, :, :, 1, :]], axis=1)
    ).reshape(nh, 128, S // 2)


def _pack_v(V):
    """[nh, S, D] -> [nh, 128, 16*65]: vx[p, 65t+d] = [V|1][128t+p, d]."""
    nh = V.shape[0]
    vf = np.concatenate(
        [V.astype(np.float16), np.ones((nh, S, 1), np.float16)], axis=-1
    )
    vf = vf.reshape(nh, S // 128, 128, D + 1).transpose(0, 2, 1, 3)
    return np.ascontiguousarray(vf).reshape(nh, 128, (S // 128) * (D + 1))


_NC_CACHE = {}


def _build_nc(n_heads=HEADS_PER_CORE, s=S):
    key = (n_heads, s)
    if key in _NC_CACHE:
        return _NC_CACHE[key]
    import concourse.tile as tile
    from concourse import bacc, mybir

    nc = bacc.Bacc(
        "TRN2", target_bir_lowering=False, debug=False, enable_asserts=False
    )
    f32 = mybir.dt.float32
    f16 = mybir.dt.float16
    ins = {
        "qt": nc.dram_tensor("qt", [n_heads, D, s], f16, kind="ExternalInput").ap(),
        # kt pre-packed on host: [128, s//2]; partitions 0-63 = even k-tiles
        # (d = p), 64-127 = odd k-tiles (d = p - 64)
        "kt": nc.dram_tensor(
            "kt", [n_heads, 128, s // 2], f16, kind="ExternalInput"
        ).ap(),
        # v pre-packed on host: [128, n_kt*65]; vx[p, 65t+d] = [V|1][128t+p, d]
        "v": nc.dram_tensor(
            "v", [n_heads, 128, (s // 128) * 65], f16, kind="ExternalInput"
        ).ap(),
        "ctri": nc.dram_tensor("ctri", [128, 128], f16, kind="ExternalInput").ap(),
    }
    outs = {
        "ot": nc.dram_tensor("ot", [n_heads, s, D], f32, kind="ExternalOutput").ap(),
    }
    with tile.TileContext(nc) as tc:
        build_attention(tc, outs, ins, n_heads=n_heads, s=s)
    nc.compile()
    _NC_CACHE[key] = nc
    return nc


def kernel(Q, K, V, mask, trace=False):
    """Full-input entry point: shards over 8 NeuronCores, returns full output."""
    from concourse.bass_utils import run_bass_kernel_spmd

    nc = _build_nc()
    tri = _make_consts()

    Qf = np.ascontiguousarray(
        Q.reshape(B * H, S, D).transpose(0, 2, 1), dtype=np.float16
    )
    Kf = _pack_kt(K.reshape(B * H, S, D))
    Vf = _pack_v(V.reshape(B * H, S, D))

    in_maps = []
    for c in range(N_CORES):
        sl = slice(c * HEADS_PER_CORE, (c + 1) * HEADS_PER_CORE)
        in_maps.append(
            {
                "qt": Qf[sl],
                "kt": Kf[sl],
                "v": Vf[sl],
                "ctri": tri,
            }
        )

    res = run_bass_kernel_spmd(nc, in_maps, core_ids=list(range(N_CORES)), trace=trace)
    ot = np.concatenate([res.results[c]["ot"] for c in range(N_CORES)], axis=0)
    out = ot.reshape(B, H, S, D)
    kernel.last_results = res
    return np.ascontiguousarray(out, dtype=np.float32)


# revision 15
# speedup vs baseline: 1.2447x; 1.2447x over previous
"""Causal attention kernel for Trainium2 (8 NeuronCores, SPMD over heads).

Problem: B=4, H=16, S=2048, D=64, fp32.
  scores = Q @ K^T / sqrt(64); causal mask; softmax (the reference's global-max
  shift cancels exactly); out = attn @ V.

Distribution: B*H = 64 heads -> 8 heads per core, embarrassingly parallel.

Per-core algorithm (per head, one full q-pass):
  - Host pre-transposes Q,K to [D,S] per head; V gets a ones-column appended.
  - QK: scoresT[k,q] = sum_d K[k,d] Q[q,d] with K-tiles stationary and Q^T
    streaming in 512-col chunks. Contraction is D=64, so even k-tiles use PE
    rows 0-63 and odd k-tiles rows 64-127 (row packing -> pairs of matmuls
    run concurrently). Scores land in rotating 1-bank PSUM tiles.
  - exp: split across ScalarE and VectorE to double elementwise throughput.
    The first 512-chunk of each k-row (contains the diagonal block) uses
    ScalarE's exact LUT exp; remaining chunks go to whichever of ScalarE /
    VectorE has less queued work.  VectorE computes exp via a one-instruction
    Schraudolph bit-trick: int16(round(s*A + B)) reinterpreted as fp16 equals
    exp(s/8) to ~2% (the systematic part cancels in softmax; end-to-end
    contribution ~0.5% rel err).  Causal masking of the diagonal block is a
    gpsimd multiply by a triangular 0/1 matrix (post-exp).
  - PV: at-stationary matmuls: acc_q[q,0:65] += at_k[:,128q:128q+128]^T @
    [V|1]-tile. Column 64 is the softmax denominator for free. Accumulators
    for all 16 q-tiles pack into 3 PSUM banks (65 f32 each); with the
    bank-wide has_written clear of start=True, only the first write into each
    bank uses start=True.  Output is directly q-major: no PE transposes.
  - Normalize: per-bank gather of rowsums -> DVE reciprocal -> per-partition
    scalar multiply on ScalarE/VectorE (balanced), DMA out [128,64] f32 tiles.
"""

import math
import sys

import numpy as np

if "/opt/trn_rl_repo" not in sys.path:
    sys.path.insert(0, "/opt/trn_rl_repo")

B, H, S, D = 4, 16, 2048, 64
N_CORES = 8
HEADS_PER_CORE = (B * H) // N_CORES  # 8
CHUNK = 512  # QK moving-chunk / PSUM score-tile width (1 bank)

# Schraudolph exp-to-fp16 constants: int16(s*A16 + B16) bit-viewed as fp16
# approximates exp(s/8).  A16 = 1024 * 0.125 * log2(e); B16 = 15360 - C with
# C = 59 centering the sawtooth error (calibrated numerically; HW convert is
# round-half-even with saturation).
A16 = 1024.0 * 0.125 * math.log2(math.e)
B16 = 15360.0 - 59.0

import os
FORCE_ENG = os.environ.get("KM_FORCE_ENG", "")  # "s"/"d" to force exp engine

# q-tile -> (acc tile index, region index): accs 0/1 hold 6 q-tiles, acc 2
# holds 4; 390/390/260 f32 per partition, one PSUM bank each.
ACC_SPLIT = (6, 6, 4)


def _acc_loc(q):
    if q < 6:
        return 0, q
    if q < 12:
        return 1, q - 6
    return 2, q - 12


def _chunks(lo, hi):
    """Split [lo, hi) at absolute multiples of CHUNK."""
    out = []
    c = lo
    while c < hi:
        w = min(hi, (c // CHUNK + 1) * CHUNK) - c
        out.append((c, w))
        c += w
    return out


def build_attention(tc, outs, ins, n_heads=HEADS_PER_CORE, s=S):
    import concourse.bass as bass
    import concourse.mybir as mybir

    nc = tc.nc
    f32 = mybir.dt.float32
    f16 = mybir.dt.float16
    i16 = mybir.dt.int16
    Exp = mybir.ActivationFunctionType.Exp
    Mult = mybir.AluOpType.mult
    Add = mybir.AluOpType.add

    qt_d, kt_d, v_d = ins["qt"], ins["kt"], ins["v"]
    tri_d = ins["ctri"]
    ot_d = outs["ot"]

    n_kt = s // 128  # 16 k-tiles
    n_pairs = n_kt // 2

    # rough per-engine queued-ns estimators for load balancing
    est = {"s": 0.0, "d": 0.0}

    def s_cost(w):
        return (172.0 + w) / 1.2

    def s_norm_cost(w):
        return (352.0 + w) / 1.2  # measured: ACTIVATE COPY ~400ns for w=64

    def d_cost(w):
        return (120.0 + w) / 0.96

    with (
        tc.tile_pool(name="consts", bufs=1) as cpool,
        tc.tile_pool(name="qpool", bufs=2) as qpool,
        tc.tile_pool(name="kpool", bufs=2) as kpool,
        tc.tile_pool(name="vpool", bufs=2) as vpool,
        tc.tile_pool(name="atpool", bufs=2) as atpool,
        tc.tile_pool(name="ofpool", bufs=8) as ofpool,
        tc.tile_pool(name="rpool", bufs=2) as rpool,
        tc.tile_pool(name="scpool", bufs=5, space="PSUM") as scpool,
        tc.tile_pool(name="accpool", bufs=1, space="PSUM") as accpool,
    ):
        c_tri = cpool.tile([128, 128], f16, tag="ctri")

        # ---- PE warm-up: ~5us of back-to-back dummy matmuls so the HAM
        # clock gate flips to 8/8 while the first head's inputs DMA in.
        # A cold PE lengthens every matmul -> exp lags -> PE stalls -> the
        # kernel locks into the throttled state; warming up front breaks it.
        wu = cpool.tile([64, CHUNK], f16, tag="wu")
        nc.vector.memset(wu[:], 0.25)
        wsc = scpool.tile([128, CHUNK], f32, tag="sc", name="warm_sc")
        for _ in range(12):
            nc.tensor.matmul(
                wsc[:, :],
                wu[:, 0:128],
                wu[:, :],
                start=True,
                stop=True,
                skip_group_check=True,
            )

        for h in range(n_heads):
            # ---- input loads (host pre-arranged to contiguous per-partition
            # lines; QK needs kt/qt first, PV needs vx only later) ----
            kt2 = kpool.tile([128, s // 2], f16, tag="kt2", name=f"kt2_{h}")
            kt2_v = kt2.rearrange("p (t c) -> p t c", c=128)
            nc.sync.dma_start(kt2[:], kt_d[h])
            qt2 = qpool.tile([128, s], f16, tag="qt2", name=f"qt2_{h}")
            nc.sync.dma_start(qt2[0:64, :], qt_d[h])
            nc.sync.dma_start(qt2[64:128, :], qt_d[h])
            vx = vpool.tile([128, n_kt * 65], f16, tag="vx", name=f"vx_{h}")
            vx_v = vx.rearrange("p (t c) -> p t c", c=65)
            nc.sync.dma_start(vx[:], v_d[h])
            if h == 0:
                nc.sync.dma_start(c_tri[:], tri_d[:])

            at_tiles = {}
            for k in range(n_kt):
                at_tiles[k] = atpool.tile(
                    [128, s - 128 * k], f16, tag=f"at{k}", name=f"at_{h}_{k}"
                )

            accs = [
                accpool.tile(
                    [128, 65 * n], f32, tag=f"acc{i}", name=f"acc{i}_{h}"
                )
                for i, n in enumerate(ACC_SPLIT)
            ]

            def qk_op(k, c0, w, p, h=h, qt2=qt2, kt2_v=kt2_v, at_tiles=at_tiles):
                """One QK chunk: matmul into a score slot + exp into at."""
                half = k % 2
                sc = scpool.tile([128, CHUNK], f32, tag="sc", name=f"sc_{h}_{k}_{c0}")
                nc.tensor.matmul(
                    sc[:, 0:w],
                    kt2_v[64 * half : 64 * half + 64, p],
                    qt2[64 * half : 64 * half + 64, c0 : c0 + w],
                    start=True,
                    stop=True,
                    skip_group_check=True,
                )
                rel = c0 - 128 * k
                first = rel == 0
                if first or FORCE_ENG == "s":
                    eng = "s"
                elif FORCE_ENG == "d":
                    eng = "d"
                else:
                    eng = "s" if est["s"] <= est["d"] else "d"
                if eng == "s":
                    nc.scalar.activation(
                        at_tiles[k][:, rel : rel + w], sc[:, 0:w], Exp, scale=0.125
                    )
                    est["s"] += s_cost(w)
                else:
                    nc.vector.tensor_scalar(
                        at_tiles[k].bitcast(i16)[:, rel : rel + w],
                        sc[:, 0:w],
                        A16,
                        B16,
                        Mult,
                        Add,
                    )
                    est["d"] += d_cost(w)
                if first:
                    # causal mask of the diagonal block (post-exp)
                    nc.gpsimd.tensor_tensor(
                        at_tiles[k][:, 0:128], at_tiles[k][:, 0:128], c_tri[:], Mult
                    )

            def qk_ops(p):
                ke, ko = 2 * p, 2 * p + 1
                ch = {ke: _chunks(128 * ke, s), ko: _chunks(128 * ko, s)}
                ops = []
                for ci in range(max(len(ch[ke]), len(ch[ko]))):
                    for k in (ke, ko):
                        if ci < len(ch[k]):
                            c0, w = ch[k][ci]
                            ops.append(lambda k=k, c0=c0, w=w: qk_op(k, c0, w, p))
                return ops

            def pv_op(k, q, at_tiles=at_tiles, accs=accs, vx_v=vx_v):
                ai, ri = _acc_loc(q)
                off = 128 * (q - k)
                nc.tensor.matmul(
                    accs[ai][:, 65 * ri : 65 * ri + 65],
                    at_tiles[k][:, off : off + 128],
                    vx_v[:, k, :],
                    start=(k == 0 and ri == 0),
                    stop=(k == q),
                    skip_group_check=True,
                )

            def pv_ops(p):
                ops = []
                for k in (2 * p, 2 * p + 1):
                    for q in range(k, n_kt):
                        ops.append(lambda k=k, q=q: pv_op(k, q))
                return ops

            def emit_norm(ai, js, h=h, accs=accs):
                """Normalize q-tiles (sum(ACC_SPLIT[:ai]) + j for j in js)."""
                q0 = sum(ACC_SPLIT[:ai])
                n = len(js)
                j0 = js[0]
                acc_v = accs[ai].rearrange("p (j c) -> p j c", c=65)
                rsum = rpool.tile(
                    [128, n], f32, tag=f"rsum{ai}_{j0}", name=f"rsum{ai}_{j0}_{h}"
                )
                nc.vector.tensor_copy(rsum[:], acc_v[:, j0 : j0 + n, 64:65])
                rcp = rpool.tile(
                    [128, n], f32, tag=f"rcp{ai}_{j0}", name=f"rcp{ai}_{j0}_{h}"
                )
                nc.vector.reciprocal(rcp[:], rsum[:])
                est["d"] += d_cost(n) + d_cost(8 * n)
                for i, j in enumerate(js):
                    q = q0 + j
                    of = ofpool.tile([128, 64], f32, tag="of", name=f"of_{h}_{q}")
                    if est["s"] + s_norm_cost(64) <= est["d"] + d_cost(64):
                        nc.scalar.mul(of[:], acc_v[:, j, 0:64], rcp[:, i : i + 1])
                        est["s"] += s_norm_cost(64)
                    else:
                        nc.vector.tensor_scalar(
                            of[:], acc_v[:, j, 0:64], rcp[:, i : i + 1], None, Mult
                        )
                        est["d"] += d_cost(64)
                    nc.sync.dma_start(ot_d[h, 128 * q : 128 * q + 128, :], of[:])

            for p in range(n_pairs):
                for op in qk_ops(p):
                    op()
                if p >= 1:
                    for op in pv_ops(p - 1):
                        op()
                if p == 3:
                    emit_norm(0, [0, 1, 2, 3, 4, 5])  # final after PV(pair 2)
                if p == 6:
                    emit_norm(1, [0, 1, 2, 3, 4, 5])  # final after PV(pair 5)
            for op in pv_ops(n_pairs - 1):
                op()
            emit_norm(2, [0, 1])
            emit_norm(2, [2, 3])


def _make_consts():
    kk, qq = np.meshgrid(np.arange(128), np.arange(128), indexing="ij")
    tri = (kk <= qq).astype(np.float16)  # keep-mask for the diagonal block
    return tri


def _pack_kt(K):
    """[nh, S, D] -> [nh, 128, S//2]: even k-tiles in partitions 0-63 (d=p),
    odd k-tiles in partitions 64-127 (d=p-64); 128-col tiles concatenated."""
    nh = K.shape[0]
    kt = K.astype(np.float16).transpose(0, 2, 1)  # [nh, D, S]
    kt = kt.reshape(nh, D, S // 256, 2, 128)
    return np.ascontiguousarray(
        np.concatenate([kt[:, :, :, 0, :], kt[:, :, :, 1, :]], axis=1)
    ).reshape(nh, 128, S // 2)


def _pack_v(V):
    """[nh, S, D] -> [nh, 128, 16*65]: vx[p, 65t+d] = [V|1][128t+p, d]."""
    nh = V.shape[0]
    vf = np.concatenate(
        [V.astype(np.float16), np.ones((nh, S, 1), np.float16)], axis=-1
    )
    vf = vf.reshape(nh, S // 128, 128, D + 1).transpose(0, 2, 1, 3)
    return np.ascontiguousarray(vf).reshape(nh, 128, (S // 128) * (D + 1))


_NC_CACHE = {}


def _build_nc(n_heads=HEADS_PER_CORE, s=S):
    key = (n_heads, s)
    if key in _NC_CACHE:
        return _NC_CACHE[key]
    import concourse.tile as tile
    from concourse import bacc, mybir

    nc = bacc.Bacc(
        "TRN2", target_bir_lowering=False, debug=False, enable_asserts=False
    )
    f32 = mybir.dt.float32
    f16 = mybir.dt.float16
    ins = {
        "qt": nc.dram_tensor("qt", [n_heads, D, s], f16, kind="ExternalInput").ap(),
        # kt pre-packed on host: [128, s//2]; partitions 0-63 = even k-tiles
        # (d = p), 64-127 = odd k-tiles (d = p - 64)
        "kt": nc.dram_tensor(
            "kt", [n_heads, 128, s // 2], f16, kind="ExternalInput"
        ).ap(),
        # v pre-packed on host: [128, n_kt*65]; vx[p, 65t+d] = [V|1][128t+p, d]
        "v": nc.dram_tensor(
            "v", [n_heads, 128, (s // 128) * 65], f16, kind="ExternalInput"
        ).ap(),
        "ctri": nc.dram_tensor("ctri", [128, 128], f16, kind="ExternalInput").ap(),
    }
    outs = {
        "ot": nc.dram_tensor("ot", [n_heads, s, D], f32, kind="ExternalOutput").ap(),
    }
    with tile.TileContext(nc) as tc:
        build_attention(tc, outs, ins, n_heads=n_heads, s=s)
    nc.compile()
    _NC_CACHE[key] = nc
    return nc


def kernel(Q, K, V, mask, trace=False):
    """Full-input entry point: shards over 8 NeuronCores, returns full output."""
    from concourse.bass_utils import run_bass_kernel_spmd

    nc = _build_nc()
    tri = _make_consts()

    Qf = np.ascontiguousarray(
        Q.reshape(B * H, S, D).transpose(0, 2, 1), dtype=np.float16
    )
    Kf = _pack_kt(K.reshape(B * H, S, D))
    Vf = _pack_v(V.reshape(B * H, S, D))

    in_maps = []
    for c in range(N_CORES):
        sl = slice(c * HEADS_PER_CORE, (c + 1) * HEADS_PER_CORE)
        in_maps.append(
            {
                "qt": Qf[sl],
                "kt": Kf[sl],
                "v": Vf[sl],
                "ctri": tri,
            }
        )

    res = run_bass_kernel_spmd(nc, in_maps, core_ids=list(range(N_CORES)), trace=trace)
    ot = np.concatenate([res.results[c]["ot"] for c in range(N_CORES)], axis=0)
    out = ot.reshape(B, H, S, D)
    kernel.last_results = res
    return np.ascontiguousarray(out, dtype=np.float32)
